# revision 16
# baseline (speedup 1.0000x reference)
"""Column-wise RMS normalization on 8 Trainium2 NeuronCores.

Computes y = x * rsqrt(sum(x*x, axis=0) + eps) for x [32768, 2048] f32.

Strategy: the tolerance (2e-2) admits fp16 I/O, so the host casts x to
fp16 and transposes it to [D, N]; each core owns 256 transposed rows
(original columns), two per partition ("(k p) t" layout). Each column
then lives entirely inside one partition with unit stride, so the
per-column sum-of-squares needs no cross-partition reduction and the
rsqrt scale is a per-partition scalar consumed by tensor_scalar_mul.

Traffic is 16MiB in + 16MiB out per core. Measured per-core HBM runs
~420 GB/s one-directional but only ~350 GB/s with reads and writes
mixed, so the kernel is deliberately SERIAL: every DMA goes through the
single sync-engine HWDGE FIFO ring, which drains all input before the
first store. The square+accumulate work is split between the vector
engine (scalar_tensor_tensor with accum_out; tensor_tensor_reduce
faults real TRN2 despite passing CoreSim) and the scalar engine
(activation Square with accum_out) so both hide under the input stream,
and the first few scaled output tiles are precomputed during the input
phase so the store stream starts the moment the last load drains.
"""

import numpy as np

import concourse.bacc as bacc
import concourse.bass as bass
import concourse.tile as tile
from concourse import mybir
from concourse.bass_utils import run_bass_kernel_spmd

N, D = 32768, 2048
EPS = 1e-6
NCORES = 8
R = D // NCORES  # 256 transposed rows (original columns) per core
P = 128          # partitions
K = R // P       # 2 column groups per core
T = N            # 32768 samples per column

# Load chunks along t (elements), per column group, 1:1 with the
# square+accumulate chunks; the tail ramps down so the last accumulate
# feeding the scale lands quickly. Chunks >= 1024 are split between the
# scalar engine (front ACT_FRAC) and the vector engine (rest) so both
# engines track the stream together and the end-of-stream tail clears
# in parallel; smaller tail chunks go whole to the vector engine, whose
# small-op overhead is ~3x lower than ACT's.
A_CHUNKS = [4096] * 7 + [2048, 1024, 512, 256, 256]
ACT_FRAC = 0.45
# store chunks: small head so the stream starts instantly, then 1MiB.
OUT_CHUNKS = [512, 512, 1024, 2048] + [4096] * 7
NCH = len(A_CHUNKS)
# accumulator slot layout per column group: 2 per split chunk, 1 per
# whole-DVE chunk
NSLOT = sum(2 if c >= 1024 else 1 for c in A_CHUNKS)
assert sum(A_CHUNKS) == T and sum(OUT_CHUNKS) == T

_NC = None


def _build() -> bass.Bass:
    nc = bacc.Bacc("TRN2", target_bir_lowering=False, enable_partition_id=False)
    x = nc.dram_tensor("x", [R, T], mybir.dt.float16, kind="ExternalInput")
    y = nc.dram_tensor("y", [R, T], mybir.dt.float16, kind="ExternalOutput")
    xv = x[:, :].rearrange("(k p) t -> p k t", k=K)
    yv = y[:, :].rearrange("(k p) t -> p k t", k=K)

    with tile.TileContext(nc) as tc:
        with (
            tc.tile_pool(name="cache", bufs=1) as cachep,
            tc.tile_pool(name="consts", bufs=1) as consts,
            tc.tile_pool(name="scr", bufs=2) as scrp,
            tc.tile_pool(name="outs", bufs=5) as outp,
        ):
            xc = cachep.tile([P, K, T], mybir.dt.float16)
            eps_t = consts.tile([P, 1], mybir.dt.float32)
            nc.vector.memset(eps_t, EPS)
            parts = consts.tile([P, K * NSLOT], mybir.dt.float32)
            u2 = consts.tile([P, K], mybir.dt.float32)
            t2 = consts.tile([P, K], mybir.dt.float32)
            s2 = consts.tile([P, K], mybir.dt.float32)

            def dma_in(k):
                t0 = 0
                for tc_ in A_CHUNKS:
                    nc.sync.dma_start(
                        out=xc[:, k, t0 : t0 + tc_], in_=xv[:, k, t0 : t0 + tc_]
                    )
                    t0 += tc_

            def sq_dve(k, src, n, slot):
                # parts[:, k*NSLOT+slot] = sum(src**2); scalar_tensor_tensor,
                # not tensor_tensor_reduce: the latter passes CoreSim but
                # faults the exec unit on real TRN2.
                scr = scrp.tile([P, 4096], mybir.dt.float16, tag="scr")
                nc.vector.scalar_tensor_tensor(
                    out=scr[:, :n],
                    in0=src,
                    scalar=1.0,
                    in1=src,
                    op0=mybir.AluOpType.mult,
                    op1=mybir.AluOpType.mult,
                    accum_out=parts[:, k * NSLOT + slot : k * NSLOT + slot + 1],
                )

            def sq_act(k, src, n, slot):
                scr = scrp.tile([P, 4096], mybir.dt.float16, tag="scra")
                nc.scalar.activation(
                    out=scr[:, :n],
                    in_=src,
                    func=mybir.ActivationFunctionType.Square,
                    accum_out=parts[:, k * NSLOT + slot : k * NSLOT + slot + 1],
                )

            def a_chunk(k, j, t0, tc_, slot):
                if tc_ >= 1024:
                    s = (int(tc_ * ACT_FRAC) + 15) // 16 * 16
                    sq_act(k, xc[:, k, t0 : t0 + s], s, slot)
                    sq_dve(k, xc[:, k, t0 + s : t0 + tc_], tc_ - s, slot + 1)
                    return slot + 2
                sq_dve(k, xc[:, k, t0 : t0 + tc_], tc_, slot)
                return slot + 1

            def scale(k):
                pv = parts[:, k * NSLOT : (k + 1) * NSLOT].rearrange(
                    "p (a j) -> p a j", a=1
                )
                nc.vector.reduce_sum(u2[:, k : k + 1], pv, axis=mybir.AxisListType.X)
                nc.scalar.activation(
                    out=t2[:, k : k + 1],
                    in_=u2[:, k : k + 1],
                    func=mybir.ActivationFunctionType.Sqrt,
                    bias=eps_t[:, :],
                    scale=1.0,
                )
                nc.vector.reciprocal_approx_fast(
                    out=s2[:, k : k + 1], in_=t2[:, k : k + 1]
                )

            out_off = [0]
            for tc_ in OUT_CHUNKS[:-1]:
                out_off.append(out_off[-1] + tc_)

            def out_mul(k, m):
                ot = outp.tile([P, 4096], mybir.dt.float16, tag="ot")
                tc_ = OUT_CHUNKS[m]
                t0 = out_off[m]
                nc.vector.tensor_scalar_mul(
                    ot[:, :tc_], xc[:, k, t0 : t0 + tc_], s2[:, k : k + 1]
                )
                return ot

            def out_dma(k, m, ot):
                tc_ = OUT_CHUNKS[m]
                t0 = out_off[m]
                nc.scalar.dma_start(out=yv[:, k, t0 : t0 + tc_], in_=ot[:, :tc_])

            # input stream on the sync ring
            dma_in(0)
            dma_in(1)
            # squares chase the stream; k0 then k1
            for k in range(K):
                t0 = 0
                slot = 0
                for j, tc_ in enumerate(A_CHUNKS):
                    slot = a_chunk(k, j, t0, tc_, slot)
                    t0 += tc_
                assert slot == NSLOT
                scale(k)
            # store stream on the scalar ring, k1 FIRST: scale1 depends on
            # the last input byte, so no store can mix with the load stream
            # (mixed-direction DMA measures ~350 GB/s vs ~420 one-way).
            # By now the scalar engine has no compute left, so its DMA
            # issues are back-to-back and every store drains immediately
            # (queued-but-undrained stores + tile reuse is a WAR race).
            for m in range(len(OUT_CHUNKS)):
                out_dma(1, m, out_mul(1, m))
            for m in range(len(OUT_CHUNKS)):
                out_dma(0, m, out_mul(0, m))
    nc.compile()
    return nc


def _get_nc() -> bass.Bass:
    global _NC
    if _NC is None:
        _NC = _build()
    return _NC


def make_in_maps(x: np.ndarray) -> list[dict]:
    xt = np.ascontiguousarray(x.T.astype(np.float16))
    return [{"x": xt[i * R : (i + 1) * R]} for i in range(NCORES)]


def kernel(x) -> np.ndarray:
    x = np.asarray(x, dtype=np.float32)
    assert x.shape == (N, D), x.shape
    nc = _get_nc()
    in_maps = make_in_maps(x)
    try:
        res = run_bass_kernel_spmd(nc, in_maps, core_ids=list(range(NCORES)))
    except Exception:
        # Transient NRT/device hiccups (e.g. a previous process's profiling
        # session left a core wedged) recover after a short pause.
        import time

        time.sleep(5)
        res = run_bass_kernel_spmd(nc, in_maps, core_ids=list(range(NCORES)))
    yt = np.concatenate([r["y"] for r in res.results], axis=0)
    return yt.T.astype(np.float32)
